# revision 50
# baseline (speedup 1.0000x reference)
"""Sharded MHA-with-RoPE Trainium2 kernel (nn_CustomTorchMHASelf).

Contract: kernel(**inputs) takes the FULL unsharded inputs of the
reference (x [2,2048,2048], Wqkv_w [6144,2048], Wqkv_b [6144],
out_w [2048,2048], out_b [2048]) and returns the full [2,2048,2048]
fp32 output, running the compute on 8 NeuronCores.

Sharding: core = b*4 + g handles batch b and head-group g (4 of the 16
heads). Each core computes q/k/v projections for its heads, RoPE,
softmax attention, and its slice of the out-projection; the host sums
the 4 partial outputs per batch and adds out_b.

Device data plane is bf16 (fp32 PSUM accumulation); the host
pre-transposes x and the weight slices into the layouts the TensorE
wants (contraction dim on partitions everywhere).

Schedule: pass 1 computes K+RoPE and V for all tokens (the last block
also hides the attention prologue under its V-projection); pass 2 is a
flat software pipeline over (block, head) steps -- at step k the PE
stream interleaves att@V(k), scores(k+1), q-projection(k+2) and a
quarter of the previous block's out-projection (65 matmuls/step), so
the ScalarE exp stream (16/step) is never on the critical path.
Key device tricks:
  - rotate-half for RoPE is a PE matmul with a signed permutation
    matrix (SBUF-SBUF DMA swaps are slow and their DIRECT2D triggers
    serialize on the sync sequencer);
  - the softmax denominator is a bf16 tree-add into the attB tile on
    VectorE plus ONE ones-matmul per (head, block) instead of 16 full
    PE ones-matmuls; att is split into two tiles (attA/attB) so the
    tree's writes never alias tiles the PE still reads (the dep
    tracker is coarse); the ones-matmul+normalize are deferred one
    step so the PE never waits on the tree;
  - 1/denominator = Exp(-Ln(d)) on ScalarE (ln and exp share an
    activation table, so no table reloads) because DVE reciprocal is
    slow and custom-DVE ops don't compile on this toolchain;
  - ~40 warm-up matmuls on the ones tile ramp the PE out of its
    1.2GHz cold p-state while the first weight/x DMAs land;
  - output tiles are written bf16, with drain-phase DMA triggers
    alternating between the scalar and sync queues.
"""

import math
import os
import sys
import types

import numpy as np
import ml_dtypes

import concourse.bass as bass
import concourse.mybir as mybir
import concourse.tile as tile
from concourse.bass import ds

F32 = mybir.dt.float32
BF16 = mybir.dt.bfloat16
Alu = mybir.AluOpType
Act = mybir.ActivationFunctionType
BF = ml_dtypes.bfloat16

S, E, HTOT, HL, D, P = 2048, 2048, 16, 4, 128, 128

# Filled with the profile exec time (ns) when MHA_TRACE=1; read by test.py.
LAST_EXEC_NS = None


def _install_axon_ntff_shim():
    """Provide antenv.axon_hooks so trace=True can reach the axon NTFF hook."""
    if "antenv.axon_hooks" in sys.modules:
        return
    mod = types.ModuleType("antenv.axon_hooks")
    holder = [None]
    mod.set_axon_ntff_profile_hook = lambda h: holder.__setitem__(0, h)
    mod.get_axon_ntff_profile_hook = lambda: holder[0]
    sys.modules["antenv.axon_hooks"] = mod
    try:
        import antenv
        antenv.axon_hooks = mod
    except ImportError:
        pass
    # boot() ran at interpreter start (sitecustomize), before this module
    # existed, so its NTFF-hook registration was silently skipped. Redo it.
    try:
        from trn_agent_boot.trn_boot import _ntff_profile_via_ctypes
        hook = _ntff_profile_via_ctypes("/opt/axon/libaxon_pjrt.so")
        if hook is not None:
            mod.set_axon_ntff_profile_hook(hook)
    except Exception:
        pass


def _split_multi_waits(nc):
    """Hoist extra sem-waits onto standalone NoOps (one wait per inst).

    This walrus build rejects any instruction carrying more than one
    sync-wait ("Too many sync wait commands"); Tile attaches one wait per
    outstanding semaphore to the consuming instruction. Splitting them
    across same-engine NoOps placed immediately before is equivalent:
    the engine executes serially, so all waits still precede the inst.
    """
    ctr = 0
    for fn in nc.m.functions:
        for blk in fn.blocks:
            out = []
            for inst in blk.instructions:
                si = getattr(inst, "sync_info", None)
                if si is not None and si.on_wait is not None \
                        and len(si.on_wait) > 1:
                    waits = list(si.on_wait)
                    si.on_wait = [waits[-1]]
                    for w in waits[:-1]:
                        ctr += 1
                        nop = mybir.InstNoOp(
                            name=f"I-wsplit-{ctr}", ins=[], outs=[])
                        nop.engine = inst.engine
                        nop.sync_info = mybir.SyncInfo(
                            on_wait=[w], on_update=[])
                        out.append(nop)
                out.append(inst)
            blk.instructions[:] = out


def _build_mha(nc: bass.Bass):
    """Emit the per-core MHA program (one shard) into `nc`."""
    EO = E // P            # contraction subtiles for the projections
    ST = 512               # free-dim tile (one PSUM bank of fp32)
    NS = S // ST
    SB = S // P
    JT = S // P            # key blocks per head
    ET = E // ST
    H = D // 2

    # packed layouts: [.., P, EO, ST] so DMA descriptors are 2KB
    # per-partition runs (1KB rows are descriptor-bound at ~half the
    # per-queue DMA bandwidth)
    xP = nc.dram_tensor("xP", [NS, P, EO, ST], BF16, kind="ExternalInput")
    wkP = nc.dram_tensor("wkP", [P, EO, HL * D], BF16, kind="ExternalInput")
    wqP = nc.dram_tensor("wqP", [P, EO, HL * D], BF16, kind="ExternalInput")
    wvP = nc.dram_tensor("wvP", [P, EO, HL * D], BF16, kind="ExternalInput")
    qkb = nc.dram_tensor("qkb", [2 * HL, D], F32, kind="ExternalInput")
    vb = nc.dram_tensor("vb", [HL * D], F32, kind="ExternalInput")
    cosT = nc.dram_tensor("cosT", [D, S], BF16, kind="ExternalInput")
    sinT = nc.dram_tensor("sinT", [D, S], BF16, kind="ExternalInput")
    owT = nc.dram_tensor("owT", [HL * D, E], BF16, kind="ExternalInput")
    ones = nc.dram_tensor("ones", [P, P], BF16, kind="ExternalInput")
    perm = nc.dram_tensor("perm", [P, P], BF16, kind="ExternalInput")
    out = nc.dram_tensor("out", [S, E], BF16, kind="ExternalOutput")

    isc = 1.0 / math.sqrt(D)

    from contextlib import ExitStack

    with tile.TileContext(nc) as tc, ExitStack() as stk:
        persist = stk.enter_context(tc.tile_pool(name="persist", bufs=1))
        kT_sb = persist.tile([P, HL, S], BF16)      # k post-RoPE [d, h, s]
        v_sb = persist.tile([P, SB, HL * D], BF16)  # v natural [s%128, s//128, hd]
        ctxT_sb = persist.tile([P, HL, S], BF16)    # [d, h, i]
        ones_sb = persist.tile([P, P], BF16)
        perm_sb = persist.tile([P, P], BF16)
        cos_sb = persist.tile([P, S], BF16)
        sin_sb = persist.tile([P, S], BF16)
        qkb_sb = persist.tile([P, 2 * HL], F32)
        vb_sb = persist.tile([P, HL * D], F32)
        ow_sb = persist.tile([P, HL, E], BF16)
        nc.sync.dma_start(ones_sb[:], ones[:])
        nc.sync.dma_start(qkb_sb[:], qkb[:].rearrange("c d -> d c"))
        nc.sync.dma_start(perm_sb[:], perm[:])

        # x stream shared by both passes; rope temps likewise.  qb/rot are
        # still being read (by the swap DMAs / mults) when the next rope
        # starts, so they get 2 bufs; t1/t2 are consumed immediately by the
        # in-order VectorE queue, so 1 buf suffices.
        xs = stk.enter_context(tc.tile_pool(name="xstream", bufs=2))
        rta = stk.enter_context(tc.tile_pool(name="ropetmpa", bufs=4))
        rtb = stk.enter_context(tc.tile_pool(name="ropetmpb", bufs=1))
        wqp = stk.enter_context(tc.tile_pool(name="wqpool", bufs=1))
        wq_sb = wqp.tile([P, EO, HL * D], BF16)

        psA = stk.enter_context(tc.tile_pool(name="psA", bufs=4, space="PSUM"))
        psS = stk.enter_context(tc.tile_pool(name="psS", bufs=2, space="PSUM"))
        psC = stk.enter_context(tc.tile_pool(name="psC", bufs=2, space="PSUM"))

        qp = stk.enter_context(tc.tile_pool(name="qpool", bufs=4))
        dp = stk.enter_context(tc.tile_pool(name="denp", bufs=1))
        oc = stk.enter_context(tc.tile_pool(name="ocopy", bufs=6))
        at0 = stk.enter_context(tc.tile_pool(name="att0p", bufs=1))

        # flat (block, head) schedule for the attention pass; blocks in
        # reverse order so the first one reuses pass 1's last x tile
        order = list(range(NS - 1, -1, -1))
        seq = [(i, h) for i in order for h in range(HL)]
        NK = len(seq)

        def blk(k):
            return seq[k][0]

        def sl_of(k):
            return ds(blk(k) * ST, ST)

        def dma_packed(dst, srcap):
            # dst [P, EO, ST] SBUF tile, srcap [P, EO, ST] DRAM view with
            # per-partition-contiguous rows: 16 transfers of 64x2KB descs
            for j in range(EO // 2):
                for ph in range(2):
                    nc.sync.dma_start(
                        dst[ds(ph * 64, 64), ds(2 * j, 2), :],
                        srcap[ds(ph * 64, 64), ds(2 * j, 2), :])

        def rope_begin(ps, bias_ap):
            # qb = q + bias (bf16 so the rotate-half matmul runs full rate)
            qb = rta.tile([P, ST], BF16, tag="qb")
            nc.vector.tensor_scalar_add(qb[:], ps[:], bias_ap)
            return qb

        def rope_finish(qb, sl, out_ap):
            # rotate-half as a PE matmul with a signed permutation matrix
            # (cross-partition moves otherwise need a slow SBUF-SBUF DMA
            # whose trigger also serializes on the sync sequencer);
            # out = qb*cos + (perm.T @ qb)*sin.
            rps = psS.tile([P, ST], F32, tag="sc")
            nc.tensor.matmul(rps[:], perm_sb[:], qb[:], start=True, stop=True)
            t1 = rtb.tile([P, ST], BF16, tag="t1")
            t2 = rtb.tile([P, ST], BF16, tag="t2")
            nc.vector.tensor_tensor(t1[:], qb[:], cos_sb[:, sl], Alu.mult)
            nc.vector.tensor_tensor(t2[:], rps[:], sin_sb[:, sl], Alu.mult)
            nc.vector.tensor_tensor(out_ap, t1[:], t2[:], Alu.add)

        def qproj_mm(k, psq, xt, eo):
            h = seq[k][1]
            nc.tensor.matmul(
                psq[:], wq_sb[:, eo, ds(h * D, D)], xt[:, eo, :],
                start=(eo == 0), stop=(eo == EO - 1))

        def qproj_fin(k, psq):
            qt = qp.tile([P, ST], BF16, tag="qt")
            qb = rope_begin(psq, qkb_sb[:, HL + seq[k][1], None])
            return qt, (qb, sl_of(k), qt[:])

        def scores_mm(k, attab, qt, jb):
            h = seq[k][1]
            ps = psS.tile([P, ST], F32, tag="sc")
            nc.tensor.matmul(
                ps[:], kT_sb[:, h, ds(jb * P, P)], qt[:],
                start=True, stop=True)
            dst = attab[jb // 8][:, jb % 8, :]
            nc.scalar.activation(dst, ps[:], Act.Exp, scale=isc)

        # PE warm-up: matmuls on the just-landed ones tile so the PE ramps
        # to full clock while the big weight/x DMAs stream in (cold matmuls
        # otherwise run at the 1.2GHz mid p-state for ~10us)
        wsink = persist.tile([P, 1], F32)
        wps = psS.tile([P, ST], F32, tag="sc", name="warmps")
        NWARM = 56
        for w in range(NWARM):
            nc.tensor.matmul(
                wps[:, :P], ones_sb[:], ones_sb[:],
                start=(w == 0), stop=(w == NWARM - 1))
        nc.vector.tensor_copy(wsink[:], wps[:, :1])

        # ---- pass 1: K projection + RoPE, V projection ----
        # The last block additionally hides the attention pass's prologue
        # (q heads 0/1 + scores/exp for head 0) under its V-projection.
        xt_last = None
        att0 = None
        qtiles = []
        with tc.tile_pool(name="p1w", bufs=1) as p1:
            wk_sb = p1.tile([P, EO, HL * D], BF16)
            wv_sb = p1.tile([P, EO, HL * D], BF16)
            # DMA priority order: the first K matmul group needs all of
            # xt0 + wk; cos/sin (bf16, 1MB) unblock the first RoPEs; then
            # x block 1, wv (needed ~25us in), and the small tables.
            xt0 = xs.tile([P, EO, ST], BF16, tag="xt", name="xt0")
            for j in range(EO // 2):
                for ph in range(2):
                    nc.sync.dma_start(
                        wk_sb[ds(ph * 64, 64), ds(2 * j, 2), :],
                        wkP[ds(ph * 64, 64), ds(2 * j, 2), :])
                    nc.sync.dma_start(
                        xt0[ds(ph * 64, 64), ds(2 * j, 2), :],
                        xP[0][ds(ph * 64, 64), ds(2 * j, 2), :])
            dma_packed(wv_sb, wvP[:])
            xt1 = xs.tile([P, EO, ST], BF16, tag="xt", name="xt1")
            dma_packed(xt1, xP[1])
            # cos/sin feed only the VectorE side of RoPE, which can lag;
            # nothing PE-side waits on them
            nc.sync.dma_start(cos_sb[:], cosT[:])
            nc.sync.dma_start(sin_sb[:], sinT[:])
            nc.sync.dma_start(vb_sb[:], vb[None, :].to_broadcast((P, HL * D)))

            for i in range(NS):
                if i == 0:
                    xt = xt0
                elif i == 1:
                    xt = xt1
                else:
                    xt = xs.tile([P, EO, ST], BF16, tag="xt")
                    dma_packed(xt, xP[i])
                    if i == NS - 1:
                        # prefetch pass-2 weights behind this block's x:
                        # wq feeds the q-projections later in this block,
                        # ow the out-projection a block later
                        dma_packed(wq_sb, wqP[:])
                        for ho in range(HL):
                            nc.sync.dma_start(
                                ow_sb[:, ho, :], owT[ds(ho * P, P), :])
                sl = ds(i * ST, ST)
                kropes = []
                for jb in range(HL):       # k head jb
                    ps = psA.tile([P, ST], F32, tag="acc")
                    for eo in range(EO):
                        nc.tensor.matmul(
                            ps[:], wk_sb[:, eo, ds(jb * D, D)], xt[:, eo, :],
                            start=(eo == 0), stop=(eo == EO - 1))
                    qb = rope_begin(ps, qkb_sb[:, jb, None])
                    kropes.append((qb, sl, kT_sb[:, jb, sl]))
                if i < NS - 1:
                    for sbl in range(ST // P):
                        sb = i * (ST // P) + sbl
                        ps = psS.tile([P, ST], F32, tag="sc")
                        for eo in range(EO):
                            nc.tensor.matmul(
                                ps[:, : HL * D], xt[:, eo, ds(sbl * P, P)],
                                wv_sb[:, eo, :],
                                start=(eo == 0), stop=(eo == EO - 1))
                        nc.vector.tensor_tensor(
                            v_sb[:, sb, :], ps[:, : HL * D], vb_sb[:], Alu.add)
                    for kr in kropes:
                        rope_finish(*kr)
                else:
                    for kr in kropes:
                        rope_finish(*kr)
                    # q-projections for the first two attention heads, then
                    # V-projection interleaved with scores/exp for head 0
                    psq0 = psA.tile([P, ST], F32, tag="acc")
                    for eo in range(EO):
                        qproj_mm(0, psq0, xt, eo)
                    qt0, rf0 = qproj_fin(0, psq0)
                    psq1 = psA.tile([P, ST], F32, tag="acc")
                    for eo in range(EO):
                        qproj_mm(1, psq1, xt, eo)
                    qt1, rf1 = qproj_fin(1, psq1)
                    qtiles.extend([qt0, qt1])
                    rope_finish(*rf0)
                    att0 = (at0.tile([P, 8, ST], BF16, name="att0A"),
                            at0.tile([P, 8, ST], BF16, name="att0B"))
                    sc_jb = 0
                    vps = None
                    for vi in range(4 * EO):
                        sbl, eo = vi // EO, vi % EO
                        if eo == 0:
                            vps = psA.tile([P, ST], F32, tag="acc")
                        nc.tensor.matmul(
                            vps[:, : HL * D], xt[:, eo, ds(sbl * P, P)],
                            wv_sb[:, eo, :],
                            start=(eo == 0), stop=(eo == EO - 1))
                        if eo == EO - 1:
                            nc.vector.tensor_tensor(
                                v_sb[:, i * (ST // P) + sbl, :],
                                vps[:, : HL * D], vb_sb[:], Alu.add)
                        if vi >= EO and (vi - EO) % 3 == 0 and sc_jb < JT:
                            scores_mm(0, att0, qt0, sc_jb)
                            sc_jb += 1
                        if vi == 40:
                            rope_finish(*rf1)
                if i == NS - 1:
                    xt_last = xt

        # ---- pass 2: flat software pipeline over (block, head) steps ----
        # Blocks run in reverse so the first one reuses pass 1's last x
        # tile.  At step k: att@V + denominator tree for head k, scores+exp
        # for head k+1, Q-projection for head k+2, one quarter of the
        # PREVIOUS block's out-projection, and the deferred denominator
        # ones-matmul + normalize for head k-1 -- all interleaved so the PE
        # stream (65 matmuls/step) hides the exp stream (16/step).
        with tc.tile_pool(name="attpa", bufs=3) as abA, \
             tc.tile_pool(name="attpb", bufs=3) as abB:

            xts = {order[0]: xt_last}

            def cblock_mm(ci, jb, pst, drain=False):
                # one of the 16x4 out-projection matmuls for token block ci;
                # jb runs 0..63 across the block's four steps
                tile_i, ho = jb // HL, jb % HL
                sb = ci * (ST // P) + tile_i // ET
                et = tile_i % ET
                if ho == 0:
                    pst[0] = psC.tile([P, ST], F32, tag="ct", name="ct")
                nc.tensor.matmul(
                    pst[0][:], ctxT_sb[:, ho, ds(sb * P, P)],
                    ow_sb[:, ho, ds(et * ST, ST)],
                    start=(ho == 0), stop=(ho == HL - 1))
                if ho == HL - 1:
                    ot = oc.tile([P, ST], BF16, tag="ot")
                    nc.vector.tensor_copy(ot[:], pst[0][:])
                    # during the drain the ScalarE queue is idle (no exp
                    # stream): alternate triggers between it and the sync
                    # queue so neither serializes the tail; split the last
                    # tiles' transfers so the final DMA is short
                    if not drain:
                        nc.sync.dma_start(
                            out[ds(sb * P, P), ds(et * ST, ST)], ot[:])
                    else:
                        eng = nc.scalar if (tile_i % 2) else nc.sync
                        if tile_i < 12:
                            eng.dma_start(
                                out[ds(sb * P, P), ds(et * ST, ST)], ot[:])
                        else:
                            hst = ST // 2
                            eng.dma_start(
                                out[ds(sb * P, P), ds(et * ST, hst)],
                                ot[:, :hst])
                            eng2 = nc.sync if (tile_i % 2) else nc.scalar
                            eng2.dma_start(
                                out[ds(sb * P, P), ds(et * ST + hst, hst)],
                                ot[:, hst:])

            def finish(k, attab, psc):
                # denominator ones-matmul + normalize for head k; deferred
                # one step so the PE reaches the ones-matmul well after the
                # VectorE tree produced attB[:, 0, :].  1/d = Exp(-Ln(d)) on
                # ScalarE (ln/exp share an activation table: no reloads).
                psd = psC.tile([P, ST], F32, tag="ct")
                nc.tensor.matmul(
                    psd[:], ones_sb[:], attab[1][:, 0, :],
                    start=True, stop=True)
                lnd = dp.tile([P, ST], F32, tag="lnd")
                nc.scalar.activation(lnd[:], psd[:], Act.Ln)
                rec = dp.tile([P, ST], F32, tag="rec")
                nc.scalar.activation(rec[:], lnd[:], Act.Exp, scale=-1.0)
                nc.vector.tensor_tensor(
                    ctxT_sb[:, seq[k][1], sl_of(k)], psc[:], rec[:], Alu.mult)

            atts = [att0]

            cpst = [None]
            pending = None
            for k in range(NK):
                i, h = seq[k]
                if h == 0 and k + 4 < NK:
                    # prefetch the x tile for the NEXT block now; the DMA
                    # has a whole block (~55us) to land
                    nxt = blk(k + 4)
                    xtn = xs.tile([P, EO, ST], BF16, tag="xt")
                    dma_packed(xtn, xP[nxt])
                    xts[nxt] = xtn
                att = atts[k]
                attA, attB = att
                ci_prev = blk(k - 4) if k >= 4 else None
                if k + 1 < NK:
                    attn = (abA.tile([P, 8, ST], BF16, tag="attA", name="attA"),
                            abB.tile([P, 8, ST], BF16, tag="attB", name="attB"))
                    atts.append(attn)
                else:
                    attn = None
                if k + 2 < NK:
                    psq = psA.tile([P, ST], F32, tag="acc")
                else:
                    psq = None
                psc = psA.tile([P, ST], F32, tag="acc")
                fin_done = False
                for idx in range(JT):
                    if attn is not None:
                        scores_mm(k + 1, attn, qtiles[k + 1], idx)
                    jb = (idx + 8) % JT     # att@V: B half first
                    avs = (attA, attB)[jb // 8][:, jb % 8, :]
                    nc.tensor.matmul(
                        psc[:], v_sb[:, jb, ds(h * D, D)], avs,
                        start=(idx == 0), stop=(idx == JT - 1))
                    if psq is not None:
                        qproj_mm(k + 2, psq, xts[blk(k + 2)], idx)
                    if ci_prev is not None:
                        if k % 4 != 0:
                            cblock_mm(ci_prev, (k % 4) * JT + idx, cpst)
                        elif idx >= 8:
                            # block-boundary step: the previous block's ctx
                            # normalize lands ~1us in, so weave its out-
                            # projection into the back half, two per slot
                            cblock_mm(ci_prev, (idx - 8) * 2, cpst)
                            cblock_mm(ci_prev, (idx - 8) * 2 + 1, cpst)
                    # denominator tree levels woven into the matmul
                    # stream; they only ever write attB, whose att@V reads
                    # all finished at idx 7
                    if idx == 7:
                        nc.vector.tensor_tensor(
                            attB[:], attB[:], attA[:], Alu.add)
                    elif idx == 9:
                        nc.vector.tensor_tensor(
                            attB[:, 0:4, :], attB[:, 0:4, :],
                            attB[:, 4:8, :], Alu.add)
                    elif idx == 10:
                        nc.vector.tensor_tensor(
                            attB[:, 0:2, :], attB[:, 0:2, :],
                            attB[:, 2:4, :], Alu.add)
                    elif idx == 11:
                        nc.vector.tensor_tensor(
                            attB[:, 0, :], attB[:, 0, :], attB[:, 1, :],
                            Alu.add)
                    elif idx == 13 and h == HL - 1:
                        # last head of a block: the tree is done (it only
                        # touches attB), so run the denominator + 1/d part
                        # of the finish chain inside the step; only the
                        # normalize mult (which must follow the full psc
                        # accumulation in emission order) remains for the
                        # step end
                        psd = psC.tile([P, ST], F32, tag="ct", name="psd")
                        nc.tensor.matmul(
                            psd[:], ones_sb[:], attB[:, 0, :],
                            start=True, stop=True)
                        lnd = dp.tile([P, ST], F32, tag="lnd")
                        nc.scalar.activation(lnd[:], psd[:], Act.Ln)
                        rec13 = dp.tile([P, ST], F32, tag="rec")
                        nc.scalar.activation(
                            rec13[:], lnd[:], Act.Exp, scale=-1.0)
                        fin_done = True
                if psq is not None:
                    qt, rf = qproj_fin(k + 2, psq)
                    qtiles.append(qt)
                else:
                    rf = None
                if pending is not None:
                    finish(*pending)
                    pending = None
                if h == HL - 1:
                    if fin_done:
                        nc.vector.tensor_tensor(
                            ctxT_sb[:, seq[k][1], sl_of(k)], psc[:],
                            rec13[:], Alu.mult)
                    else:
                        finish(k, att, psc)
                else:
                    pending = (k, att, psc)
                if rf is not None:
                    rope_finish(*rf)

            # the last block's out-projection has no next block to hide in
            cpst = [None]
            for jb in range(4 * JT):
                cblock_mm(blk(NK - 1), jb, cpst, drain=True)

    return nc


def _rope_tables():
    inv_freq = 1.0 / (10000.0 ** (np.arange(0, D, 2, dtype=np.float32) / D))
    t = np.arange(S, dtype=np.float32)
    freqs = np.einsum("s,f->sf", t, inv_freq)
    emb = np.concatenate([freqs, freqs], axis=-1)
    cosT = np.cos(emb).astype(np.float32).T.copy()
    sinT = np.sin(emb).astype(np.float32).T.copy()
    # rotate-half sign lives in the on-device permutation matrix
    return cosT.astype(BF), np.ascontiguousarray(sinT).astype(BF)


def _core_inputs(x, Wqkv_w, Wqkv_b, out_w, b, g, cosT, sinT, xT_bf):
    # k-head columns first, then q-head columns (matches kernel layout)
    k_cols, q_cols, kb_rows, qb_rows = [], [], [], []
    for hl in range(HL):
        h = g * HL + hl
        q_cols.append(Wqkv_w[h * D:(h + 1) * D, :].T)
        k_cols.append(Wqkv_w[E + h * D:E + (h + 1) * D, :].T)
        qb_rows.append(Wqkv_b[h * D:(h + 1) * D])
        kb_rows.append(Wqkv_b[E + h * D:E + (h + 1) * D])
    def pack(wT):
        # [E, HL*D] -> [P, EO, HL*D]: per-partition contiguous rows so
        # the on-device DMA uses 2KB descriptors
        return np.ascontiguousarray(
            wT.reshape(E // P, P, HL * D).transpose(1, 0, 2)).astype(BF)

    wkP = pack(np.concatenate(k_cols, axis=1))
    wqP = pack(np.concatenate(q_cols, axis=1))
    qkb = np.stack(kb_rows + qb_rows).astype(np.float32)
    v0 = 2 * E + g * HL * D
    wvP = pack(Wqkv_w[v0:v0 + HL * D, :].T)
    vb = Wqkv_b[v0:v0 + HL * D].astype(np.float32)
    owT = np.ascontiguousarray(
        out_w[:, g * HL * D:(g + 1) * HL * D].T).astype(BF)
    # rotate-half permutation: out[d] = -q[d+64] (d<64), +q[d-64] (d>=64)
    perm = np.zeros((P, P), dtype=np.float32)
    for d in range(D // 2):
        perm[d + D // 2, d] = -1.0
        perm[d, d + D // 2] = 1.0
    return {"xP": xT_bf, "wkP": wkP, "wqP": wqP, "wvP": wvP, "qkb": qkb,
            "vb": vb, "cosT": cosT, "sinT": sinT, "owT": owT,
            "ones": np.ones((P, P), BF), "perm": perm.astype(BF)}


def kernel(x, Wqkv_w, Wqkv_b, out_w, out_b):
    global LAST_EXEC_NS
    _install_axon_ntff_shim()
    from concourse.bass_utils import run_bass_kernel_spmd

    x = np.asarray(x, dtype=np.float32)
    Wqkv_w = np.asarray(Wqkv_w, dtype=np.float32)
    Wqkv_b = np.asarray(Wqkv_b, dtype=np.float32)
    out_w = np.asarray(out_w, dtype=np.float32)
    out_b = np.asarray(out_b, dtype=np.float32)

    cosT, sinT = _rope_tables()
    # x packed as [NS, P, EO, ST]: xP[i, p, eo, s] = x[b, i*ST+s, eo*P+p]
    NS, EO, ST = S // 512, E // P, 512
    xT_bf = [np.ascontiguousarray(
        x[b].reshape(NS, ST, EO, P).transpose(0, 3, 2, 1)).astype(BF)
        for b in range(2)]
    in_maps = []
    for core in range(8):
        b, g = core // 4, core % 4
        in_maps.append(
            _core_inputs(x, Wqkv_w, Wqkv_b, out_w, b, g, cosT, sinT, xT_bf[b]))

    nc = bass.Bass()
    _build_mha(nc)
    _split_multi_waits(nc)

    trace = bool(os.environ.get("MHA_TRACE"))
    if trace:
        # dev-only profiling path; skip the S3 artifact upload
        import concourse.bass_utils as _bu
        _bu.upload_artifacts = lambda tmpdir: tmpdir
    res = run_bass_kernel_spmd(
        nc, in_maps, core_ids=list(range(8)), trace=trace)
    if trace:
        LAST_EXEC_NS = res.exec_time_ns

    out = np.empty((2, S, E), dtype=np.float32)
    for b in range(2):
        acc = res.results[b * 4 + 0]["out"].astype(np.float32)
        for g in range(1, 4):
            acc += res.results[b * 4 + g]["out"].astype(np.float32)
        out[b] = acc + out_b[None, :]
    return out


# revision 51
# speedup vs baseline: 1.0073x; 1.0073x over previous
"""Sharded MHA-with-RoPE Trainium2 kernel (nn_CustomTorchMHASelf).

Contract: kernel(**inputs) takes the FULL unsharded inputs of the
reference (x [2,2048,2048], Wqkv_w [6144,2048], Wqkv_b [6144],
out_w [2048,2048], out_b [2048]) and returns the full [2,2048,2048]
fp32 output, running the compute on 8 NeuronCores.

Sharding: core = b*4 + g handles batch b and head-group g (4 of the 16
heads). Each core computes q/k/v projections for its heads, RoPE,
softmax attention, and its slice of the out-projection; the host sums
the 4 partial outputs per batch and adds out_b.

Device data plane is bf16 (fp32 PSUM accumulation); the host
pre-transposes x and the weight slices into the layouts the TensorE
wants (contraction dim on partitions everywhere).

Schedule: pass 1 computes K+RoPE and V for all tokens (the last block
also hides the attention prologue under its V-projection); pass 2 is a
flat software pipeline over (block, head) steps -- at step k the PE
stream interleaves att@V(k), scores(k+1), q-projection(k+2) and a
quarter of the previous block's out-projection (65 matmuls/step), so
the ScalarE exp stream (16/step) is never on the critical path.
Key device tricks:
  - rotate-half for RoPE is a PE matmul with a signed permutation
    matrix (SBUF-SBUF DMA swaps are slow and their DIRECT2D triggers
    serialize on the sync sequencer);
  - the softmax denominator is a bf16 tree-add into the attB tile on
    VectorE plus ONE ones-matmul per (head, block) instead of 16 full
    PE ones-matmuls; att is split into two tiles (attA/attB) so the
    tree's writes never alias tiles the PE still reads (the dep
    tracker is coarse); the ones-matmul+normalize are deferred one
    step so the PE never waits on the tree;
  - 1/denominator = Exp(-Ln(d)) on ScalarE (ln and exp share an
    activation table, so no table reloads) because DVE reciprocal is
    slow and custom-DVE ops don't compile on this toolchain;
  - ~40 warm-up matmuls on the ones tile ramp the PE out of its
    1.2GHz cold p-state while the first weight/x DMAs land;
  - output tiles are written bf16, with drain-phase DMA triggers
    alternating between the scalar and sync queues.
"""

import math
import os
import sys
import types

import numpy as np
import ml_dtypes

import concourse.bass as bass
import concourse.mybir as mybir
import concourse.tile as tile
from concourse.bass import ds

F32 = mybir.dt.float32
BF16 = mybir.dt.bfloat16
Alu = mybir.AluOpType
Act = mybir.ActivationFunctionType
BF = ml_dtypes.bfloat16

S, E, HTOT, HL, D, P = 2048, 2048, 16, 4, 128, 128

# Filled with the profile exec time (ns) when MHA_TRACE=1; read by test.py.
LAST_EXEC_NS = None


def _install_axon_ntff_shim():
    """Provide antenv.axon_hooks so trace=True can reach the axon NTFF hook."""
    if "antenv.axon_hooks" in sys.modules:
        return
    mod = types.ModuleType("antenv.axon_hooks")
    holder = [None]
    mod.set_axon_ntff_profile_hook = lambda h: holder.__setitem__(0, h)
    mod.get_axon_ntff_profile_hook = lambda: holder[0]
    sys.modules["antenv.axon_hooks"] = mod
    try:
        import antenv
        antenv.axon_hooks = mod
    except ImportError:
        pass
    # boot() ran at interpreter start (sitecustomize), before this module
    # existed, so its NTFF-hook registration was silently skipped. Redo it.
    try:
        from trn_agent_boot.trn_boot import _ntff_profile_via_ctypes
        hook = _ntff_profile_via_ctypes("/opt/axon/libaxon_pjrt.so")
        if hook is not None:
            mod.set_axon_ntff_profile_hook(hook)
    except Exception:
        pass


def _split_multi_waits(nc):
    """Hoist extra sem-waits onto standalone NoOps (one wait per inst).

    This walrus build rejects any instruction carrying more than one
    sync-wait ("Too many sync wait commands"); Tile attaches one wait per
    outstanding semaphore to the consuming instruction. Splitting them
    across same-engine NoOps placed immediately before is equivalent:
    the engine executes serially, so all waits still precede the inst.
    """
    ctr = 0
    for fn in nc.m.functions:
        for blk in fn.blocks:
            out = []
            for inst in blk.instructions:
                si = getattr(inst, "sync_info", None)
                if si is not None and si.on_wait is not None \
                        and len(si.on_wait) > 1:
                    waits = list(si.on_wait)
                    si.on_wait = [waits[-1]]
                    for w in waits[:-1]:
                        ctr += 1
                        nop = mybir.InstNoOp(
                            name=f"I-wsplit-{ctr}", ins=[], outs=[])
                        nop.engine = inst.engine
                        nop.sync_info = mybir.SyncInfo(
                            on_wait=[w], on_update=[])
                        out.append(nop)
                out.append(inst)
            blk.instructions[:] = out


def _build_mha(nc: bass.Bass):
    """Emit the per-core MHA program (one shard) into `nc`."""
    EO = E // P            # contraction subtiles for the projections
    ST = 512               # free-dim tile (one PSUM bank of fp32)
    NS = S // ST
    SB = S // P
    JT = S // P            # key blocks per head
    ET = E // ST
    H = D // 2

    # packed layouts: [.., P, EO, ST] so DMA descriptors are 2KB
    # per-partition runs (1KB rows are descriptor-bound at ~half the
    # per-queue DMA bandwidth)
    xP = nc.dram_tensor("xP", [NS, P, EO, ST], BF16, kind="ExternalInput")
    wkP = nc.dram_tensor("wkP", [P, EO, HL * D], BF16, kind="ExternalInput")
    wqP = nc.dram_tensor("wqP", [P, EO, HL * D], BF16, kind="ExternalInput")
    wvP = nc.dram_tensor("wvP", [P, EO, HL * D], BF16, kind="ExternalInput")
    qkb = nc.dram_tensor("qkb", [2 * HL, D], F32, kind="ExternalInput")
    vb = nc.dram_tensor("vb", [HL * D], F32, kind="ExternalInput")
    cosT = nc.dram_tensor("cosT", [D, S], BF16, kind="ExternalInput")
    sinT = nc.dram_tensor("sinT", [D, S], BF16, kind="ExternalInput")
    owT = nc.dram_tensor("owT", [HL * D, E], BF16, kind="ExternalInput")
    ones = nc.dram_tensor("ones", [P, P], BF16, kind="ExternalInput")
    perm = nc.dram_tensor("perm", [P, P], BF16, kind="ExternalInput")
    out = nc.dram_tensor("out", [S, E], BF16, kind="ExternalOutput")

    isc = 1.0 / math.sqrt(D)

    from contextlib import ExitStack

    with tile.TileContext(nc) as tc, ExitStack() as stk:
        persist = stk.enter_context(tc.tile_pool(name="persist", bufs=1))
        kT_sb = persist.tile([P, HL, S], BF16)      # k post-RoPE [d, h, s]
        v_sb = persist.tile([P, SB, HL * D], BF16)  # v natural [s%128, s//128, hd]
        ctxT_sb = persist.tile([P, HL, S], BF16)    # [d, h, i]
        ones_sb = persist.tile([P, P], BF16)
        perm_sb = persist.tile([P, P], BF16)
        cos_sb = persist.tile([P, S], BF16)
        sin_sb = persist.tile([P, S], BF16)
        qkb_sb = persist.tile([P, 2 * HL], F32)
        vb_sb = persist.tile([P, HL * D], F32)
        ow_sb = persist.tile([P, HL, E], BF16)
        nc.sync.dma_start(ones_sb[:], ones[:])
        nc.sync.dma_start(qkb_sb[:], qkb[:].rearrange("c d -> d c"))
        nc.sync.dma_start(perm_sb[:], perm[:])

        # x stream shared by both passes; rope temps likewise.  qb/rot are
        # still being read (by the swap DMAs / mults) when the next rope
        # starts, so they get 2 bufs; t1/t2 are consumed immediately by the
        # in-order VectorE queue, so 1 buf suffices.
        xs = stk.enter_context(tc.tile_pool(name="xstream", bufs=2))
        rta = stk.enter_context(tc.tile_pool(name="ropetmpa", bufs=4))
        rtb = stk.enter_context(tc.tile_pool(name="ropetmpb", bufs=1))
        wqp = stk.enter_context(tc.tile_pool(name="wqpool", bufs=1))
        wq_sb = wqp.tile([P, EO, HL * D], BF16)

        psA = stk.enter_context(tc.tile_pool(name="psA", bufs=4, space="PSUM"))
        psS = stk.enter_context(tc.tile_pool(name="psS", bufs=2, space="PSUM"))
        psC = stk.enter_context(tc.tile_pool(name="psC", bufs=2, space="PSUM"))

        qp = stk.enter_context(tc.tile_pool(name="qpool", bufs=4))
        dp = stk.enter_context(tc.tile_pool(name="denp", bufs=1))
        oc = stk.enter_context(tc.tile_pool(name="ocopy", bufs=6))
        at0 = stk.enter_context(tc.tile_pool(name="att0p", bufs=1))

        # flat (block, head) schedule for the attention pass; blocks in
        # reverse order so the first one reuses pass 1's last x tile
        order = list(range(NS - 1, -1, -1))
        seq = [(i, h) for i in order for h in range(HL)]
        NK = len(seq)

        def blk(k):
            return seq[k][0]

        def sl_of(k):
            return ds(blk(k) * ST, ST)

        def dma_packed(dst, srcap):
            # dst [P, EO, ST] SBUF tile, srcap [P, EO, ST] DRAM view with
            # per-partition-contiguous rows: 16 transfers of 64x2KB descs
            for j in range(EO // 2):
                for ph in range(2):
                    nc.sync.dma_start(
                        dst[ds(ph * 64, 64), ds(2 * j, 2), :],
                        srcap[ds(ph * 64, 64), ds(2 * j, 2), :])

        def rope_begin(ps, bias_ap):
            # qb = q + bias (bf16 so the rotate-half matmul runs full rate)
            qb = rta.tile([P, ST], BF16, tag="qb")
            nc.vector.tensor_scalar_add(qb[:], ps[:], bias_ap)
            return qb

        def rope_finish(qb, sl, out_ap):
            # rotate-half as a PE matmul with a signed permutation matrix
            # (cross-partition moves otherwise need a slow SBUF-SBUF DMA
            # whose trigger also serializes on the sync sequencer);
            # out = qb*cos + (perm.T @ qb)*sin.
            rps = psS.tile([P, ST], F32, tag="sc")
            nc.tensor.matmul(rps[:], perm_sb[:], qb[:], start=True, stop=True)
            t1 = rtb.tile([P, ST], BF16, tag="t1")
            t2 = rtb.tile([P, ST], BF16, tag="t2")
            nc.vector.tensor_tensor(t1[:], qb[:], cos_sb[:, sl], Alu.mult)
            nc.vector.tensor_tensor(t2[:], rps[:], sin_sb[:, sl], Alu.mult)
            nc.vector.tensor_tensor(out_ap, t1[:], t2[:], Alu.add)

        def qproj_mm(k, psq, xt, eo):
            h = seq[k][1]
            nc.tensor.matmul(
                psq[:], wq_sb[:, eo, ds(h * D, D)], xt[:, eo, :],
                start=(eo == 0), stop=(eo == EO - 1))

        def qproj_fin(k, psq):
            qt = qp.tile([P, ST], BF16, tag="qt")
            qb = rope_begin(psq, qkb_sb[:, HL + seq[k][1], None])
            return qt, (qb, sl_of(k), qt[:])

        def scores_mm(k, attab, qt, jb):
            h = seq[k][1]
            ps = psS.tile([P, ST], F32, tag="sc")
            nc.tensor.matmul(
                ps[:], kT_sb[:, h, ds(jb * P, P)], qt[:],
                start=True, stop=True)
            dst = attab[jb // 8][:, jb % 8, :]
            nc.scalar.activation(dst, ps[:], Act.Exp, scale=isc)

        # PE warm-up: matmuls on the just-landed ones tile so the PE ramps
        # to full clock while the big weight/x DMAs stream in (cold matmuls
        # otherwise run at the 1.2GHz mid p-state for ~10us)
        wsink = persist.tile([P, 1], F32)
        wps = psS.tile([P, ST], F32, tag="sc", name="warmps")
        NWARM = 56
        for w in range(NWARM):
            nc.tensor.matmul(
                wps[:, :P], ones_sb[:], ones_sb[:],
                start=(w == 0), stop=(w == NWARM - 1))
        nc.vector.tensor_copy(wsink[:], wps[:, :1])

        # ---- pass 1: K projection + RoPE, V projection ----
        # The last block additionally hides the attention pass's prologue
        # (q heads 0/1 + scores/exp for head 0) under its V-projection.
        xt_last = None
        att0 = None
        qtiles = []
        with tc.tile_pool(name="p1w", bufs=1) as p1:
            wk_sb = p1.tile([P, EO, HL * D], BF16)
            wv_sb = p1.tile([P, EO, HL * D], BF16)
            # DMA priority order: the first K matmul group needs all of
            # xt0 + wk; cos/sin (bf16, 1MB) unblock the first RoPEs; then
            # x block 1, wv (needed ~25us in), and the small tables.
            xt0 = xs.tile([P, EO, ST], BF16, tag="xt", name="xt0")
            for j in range(EO // 2):
                for ph in range(2):
                    nc.sync.dma_start(
                        wk_sb[ds(ph * 64, 64), ds(2 * j, 2), :],
                        wkP[ds(ph * 64, 64), ds(2 * j, 2), :])
                    nc.sync.dma_start(
                        xt0[ds(ph * 64, 64), ds(2 * j, 2), :],
                        xP[0][ds(ph * 64, 64), ds(2 * j, 2), :])
            dma_packed(wv_sb, wvP[:])
            xt1 = xs.tile([P, EO, ST], BF16, tag="xt", name="xt1")
            dma_packed(xt1, xP[1])
            # cos/sin feed only the VectorE side of RoPE, which can lag;
            # nothing PE-side waits on them
            nc.sync.dma_start(cos_sb[:], cosT[:])
            nc.sync.dma_start(sin_sb[:], sinT[:])
            nc.sync.dma_start(vb_sb[:], vb[None, :].to_broadcast((P, HL * D)))

            for i in range(NS):
                if i == 0:
                    xt = xt0
                elif i == 1:
                    xt = xt1
                else:
                    xt = xs.tile([P, EO, ST], BF16, tag="xt")
                    dma_packed(xt, xP[i])
                    if i == NS - 1:
                        # prefetch pass-2 weights behind this block's x:
                        # wq feeds the q-projections later in this block,
                        # ow the out-projection a block later
                        dma_packed(wq_sb, wqP[:])
                        for ho in range(HL):
                            nc.sync.dma_start(
                                ow_sb[:, ho, :], owT[ds(ho * P, P), :])
                sl = ds(i * ST, ST)
                kropes = []
                for jb in range(HL):       # k head jb
                    ps = psA.tile([P, ST], F32, tag="acc")
                    for eo in range(EO):
                        nc.tensor.matmul(
                            ps[:], wk_sb[:, eo, ds(jb * D, D)], xt[:, eo, :],
                            start=(eo == 0), stop=(eo == EO - 1))
                    qb = rope_begin(ps, qkb_sb[:, jb, None])
                    kropes.append((qb, sl, kT_sb[:, jb, sl]))
                if i < NS - 1:
                    for sbl in range(ST // P):
                        sb = i * (ST // P) + sbl
                        ps = psS.tile([P, ST], F32, tag="sc")
                        for eo in range(EO):
                            nc.tensor.matmul(
                                ps[:, : HL * D], xt[:, eo, ds(sbl * P, P)],
                                wv_sb[:, eo, :],
                                start=(eo == 0), stop=(eo == EO - 1))
                        nc.vector.tensor_tensor(
                            v_sb[:, sb, :], ps[:, : HL * D], vb_sb[:], Alu.add)
                    for kr in kropes:
                        rope_finish(*kr)
                else:
                    for kr in kropes:
                        rope_finish(*kr)
                    # q-projections for the first two attention heads, then
                    # V-projection interleaved with scores/exp for head 0
                    psq0 = psA.tile([P, ST], F32, tag="acc")
                    for eo in range(EO):
                        qproj_mm(0, psq0, xt, eo)
                    qt0, rf0 = qproj_fin(0, psq0)
                    psq1 = psA.tile([P, ST], F32, tag="acc")
                    for eo in range(EO):
                        qproj_mm(1, psq1, xt, eo)
                    qt1, rf1 = qproj_fin(1, psq1)
                    qtiles.extend([qt0, qt1])
                    rope_finish(*rf0)
                    att0 = (at0.tile([P, 8, ST], BF16, name="att0A"),
                            at0.tile([P, 8, ST], BF16, name="att0B"))
                    sc_jb = 0
                    vps = None
                    for vi in range(4 * EO):
                        sbl, eo = vi // EO, vi % EO
                        if eo == 0:
                            vps = psA.tile([P, ST], F32, tag="acc")
                        nc.tensor.matmul(
                            vps[:, : HL * D], xt[:, eo, ds(sbl * P, P)],
                            wv_sb[:, eo, :],
                            start=(eo == 0), stop=(eo == EO - 1))
                        if eo == EO - 1:
                            nc.vector.tensor_tensor(
                                v_sb[:, i * (ST // P) + sbl, :],
                                vps[:, : HL * D], vb_sb[:], Alu.add)
                        if vi >= EO and (vi - EO) % 3 == 0 and sc_jb < JT:
                            scores_mm(0, att0, qt0, sc_jb)
                            sc_jb += 1
                        if vi == 40:
                            rope_finish(*rf1)
                if i == NS - 1:
                    xt_last = xt

        # ---- pass 2: flat software pipeline over (block, head) steps ----
        # Blocks run in reverse so the first one reuses pass 1's last x
        # tile.  At step k: att@V + denominator tree for head k, scores+exp
        # for head k+1, Q-projection for head k+2, one quarter of the
        # PREVIOUS block's out-projection, and the deferred denominator
        # ones-matmul + normalize for head k-1 -- all interleaved so the PE
        # stream (65 matmuls/step) hides the exp stream (16/step).
        with tc.tile_pool(name="attpa", bufs=3) as abA, \
             tc.tile_pool(name="attpb", bufs=3) as abB:

            xts = {order[0]: xt_last}

            def cblock_mm(ci, jb, pst, drain=False):
                # one of the 16x4 out-projection matmuls for token block ci;
                # jb runs 0..63 across the block's four steps
                tile_i, ho = jb // HL, jb % HL
                sb = ci * (ST // P) + tile_i // ET
                et = tile_i % ET
                if ho == 0:
                    pst[0] = psC.tile([P, ST], F32, tag="ct", name="ct")
                nc.tensor.matmul(
                    pst[0][:], ctxT_sb[:, ho, ds(sb * P, P)],
                    ow_sb[:, ho, ds(et * ST, ST)],
                    start=(ho == 0), stop=(ho == HL - 1))
                if ho == HL - 1:
                    ot = oc.tile([P, ST], BF16, tag="ot")
                    nc.vector.tensor_copy(ot[:], pst[0][:])
                    # during the drain the ScalarE queue is idle (no exp
                    # stream): alternate triggers between it and the sync
                    # queue so neither serializes the tail; split the last
                    # tiles' transfers so the final DMA is short
                    if not drain:
                        nc.sync.dma_start(
                            out[ds(sb * P, P), ds(et * ST, ST)], ot[:])
                    else:
                        eng = nc.scalar if (tile_i % 2) else nc.sync
                        if tile_i < 12:
                            eng.dma_start(
                                out[ds(sb * P, P), ds(et * ST, ST)], ot[:])
                        else:
                            hst = ST // 2
                            eng.dma_start(
                                out[ds(sb * P, P), ds(et * ST, hst)],
                                ot[:, :hst])
                            eng2 = nc.sync if (tile_i % 2) else nc.scalar
                            eng2.dma_start(
                                out[ds(sb * P, P), ds(et * ST + hst, hst)],
                                ot[:, hst:])

            def finish(k, attab, psc):
                # denominator ones-matmul + normalize for head k; deferred
                # one step so the PE reaches the ones-matmul well after the
                # VectorE tree produced attB[:, 0, :].  1/d = Exp(-Ln(d)) on
                # ScalarE (ln/exp share an activation table: no reloads).
                psd = psC.tile([P, ST], F32, tag="ct")
                nc.tensor.matmul(
                    psd[:], ones_sb[:], attab[1][:, 0, :],
                    start=True, stop=True)
                lnd = dp.tile([P, ST], F32, tag="lnd")
                nc.scalar.activation(lnd[:], psd[:], Act.Ln)
                rec = dp.tile([P, ST], F32, tag="rec")
                nc.scalar.activation(rec[:], lnd[:], Act.Exp, scale=-1.0)
                nc.vector.tensor_tensor(
                    ctxT_sb[:, seq[k][1], sl_of(k)], psc[:], rec[:], Alu.mult)

            atts = [att0]

            cpst = [None]
            pending = None
            for k in range(NK):
                i, h = seq[k]
                if h == 0 and k + 4 < NK:
                    # prefetch the x tile for the NEXT block now; the DMA
                    # has a whole block (~55us) to land
                    nxt = blk(k + 4)
                    xtn = xs.tile([P, EO, ST], BF16, tag="xt")
                    dma_packed(xtn, xP[nxt])
                    xts[nxt] = xtn
                att = atts[k]
                attA, attB = att
                ci_prev = blk(k - 4) if k >= 4 else None
                if k + 1 < NK:
                    attn = (abA.tile([P, 8, ST], BF16, tag="attA", name="attA"),
                            abB.tile([P, 8, ST], BF16, tag="attB", name="attB"))
                    atts.append(attn)
                else:
                    attn = None
                if k + 2 < NK:
                    psq = psA.tile([P, ST], F32, tag="acc")
                else:
                    psq = None
                psc = psA.tile([P, ST], F32, tag="acc")
                for idx in range(JT):
                    if attn is not None:
                        scores_mm(k + 1, attn, qtiles[k + 1], idx)
                    jb = (idx + 8) % JT     # att@V: B half first
                    avs = (attA, attB)[jb // 8][:, jb % 8, :]
                    nc.tensor.matmul(
                        psc[:], v_sb[:, jb, ds(h * D, D)], avs,
                        start=(idx == 0), stop=(idx == JT - 1))
                    if psq is not None:
                        qproj_mm(k + 2, psq, xts[blk(k + 2)], idx)
                    if ci_prev is not None:
                        if k % 4 != 0:
                            cblock_mm(ci_prev, (k % 4) * JT + idx, cpst)
                        elif idx >= 8:
                            # block-boundary step: the previous block's ctx
                            # normalize lands ~1us in, so weave its out-
                            # projection into the back half, two per slot
                            cblock_mm(ci_prev, (idx - 8) * 2, cpst)
                            cblock_mm(ci_prev, (idx - 8) * 2 + 1, cpst)
                    # denominator tree levels woven into the matmul
                    # stream; they only ever write attB, whose att@V reads
                    # all finished at idx 7
                    if idx == 7:
                        nc.vector.tensor_tensor(
                            attB[:], attB[:], attA[:], Alu.add)
                    elif idx == 11:
                        nc.vector.tensor_tensor(
                            attB[:, 0:4, :], attB[:, 0:4, :],
                            attB[:, 4:8, :], Alu.add)
                    elif idx == 13:
                        nc.vector.tensor_tensor(
                            attB[:, 0:2, :], attB[:, 0:2, :],
                            attB[:, 2:4, :], Alu.add)
                    elif idx == 15:
                        nc.vector.tensor_tensor(
                            attB[:, 0, :], attB[:, 0, :], attB[:, 1, :],
                            Alu.add)
                if psq is not None:
                    qt, rf = qproj_fin(k + 2, psq)
                    qtiles.append(qt)
                else:
                    rf = None
                if pending is not None:
                    finish(*pending)
                    pending = None
                if h == HL - 1:
                    # last head of the block: finish NOW so the next
                    # block's interleaved out-projection reads final ctx
                    finish(k, att, psc)
                else:
                    pending = (k, att, psc)
                if rf is not None:
                    rope_finish(*rf)

            # the last block's out-projection has no next block to hide in
            cpst = [None]
            for jb in range(4 * JT):
                cblock_mm(blk(NK - 1), jb, cpst, drain=True)

    return nc


def _rope_tables():
    inv_freq = 1.0 / (10000.0 ** (np.arange(0, D, 2, dtype=np.float32) / D))
    t = np.arange(S, dtype=np.float32)
    freqs = np.einsum("s,f->sf", t, inv_freq)
    emb = np.concatenate([freqs, freqs], axis=-1)
    cosT = np.cos(emb).astype(np.float32).T.copy()
    sinT = np.sin(emb).astype(np.float32).T.copy()
    # rotate-half sign lives in the on-device permutation matrix
    return cosT.astype(BF), np.ascontiguousarray(sinT).astype(BF)


def _core_inputs(x, Wqkv_w, Wqkv_b, out_w, b, g, cosT, sinT, xT_bf):
    # k-head columns first, then q-head columns (matches kernel layout)
    k_cols, q_cols, kb_rows, qb_rows = [], [], [], []
    for hl in range(HL):
        h = g * HL + hl
        q_cols.append(Wqkv_w[h * D:(h + 1) * D, :].T)
        k_cols.append(Wqkv_w[E + h * D:E + (h + 1) * D, :].T)
        qb_rows.append(Wqkv_b[h * D:(h + 1) * D])
        kb_rows.append(Wqkv_b[E + h * D:E + (h + 1) * D])
    def pack(wT):
        # [E, HL*D] -> [P, EO, HL*D]: per-partition contiguous rows so
        # the on-device DMA uses 2KB descriptors
        return np.ascontiguousarray(
            wT.reshape(E // P, P, HL * D).transpose(1, 0, 2)).astype(BF)

    wkP = pack(np.concatenate(k_cols, axis=1))
    wqP = pack(np.concatenate(q_cols, axis=1))
    qkb = np.stack(kb_rows + qb_rows).astype(np.float32)
    v0 = 2 * E + g * HL * D
    wvP = pack(Wqkv_w[v0:v0 + HL * D, :].T)
    vb = Wqkv_b[v0:v0 + HL * D].astype(np.float32)
    owT = np.ascontiguousarray(
        out_w[:, g * HL * D:(g + 1) * HL * D].T).astype(BF)
    # rotate-half permutation: out[d] = -q[d+64] (d<64), +q[d-64] (d>=64)
    perm = np.zeros((P, P), dtype=np.float32)
    for d in range(D // 2):
        perm[d + D // 2, d] = -1.0
        perm[d, d + D // 2] = 1.0
    return {"xP": xT_bf, "wkP": wkP, "wqP": wqP, "wvP": wvP, "qkb": qkb,
            "vb": vb, "cosT": cosT, "sinT": sinT, "owT": owT,
            "ones": np.ones((P, P), BF), "perm": perm.astype(BF)}


def kernel(x, Wqkv_w, Wqkv_b, out_w, out_b):
    global LAST_EXEC_NS
    _install_axon_ntff_shim()
    from concourse.bass_utils import run_bass_kernel_spmd

    x = np.asarray(x, dtype=np.float32)
    Wqkv_w = np.asarray(Wqkv_w, dtype=np.float32)
    Wqkv_b = np.asarray(Wqkv_b, dtype=np.float32)
    out_w = np.asarray(out_w, dtype=np.float32)
    out_b = np.asarray(out_b, dtype=np.float32)

    cosT, sinT = _rope_tables()
    # x packed as [NS, P, EO, ST]: xP[i, p, eo, s] = x[b, i*ST+s, eo*P+p]
    NS, EO, ST = S // 512, E // P, 512
    xT_bf = [np.ascontiguousarray(
        x[b].reshape(NS, ST, EO, P).transpose(0, 3, 2, 1)).astype(BF)
        for b in range(2)]
    in_maps = []
    for core in range(8):
        b, g = core // 4, core % 4
        in_maps.append(
            _core_inputs(x, Wqkv_w, Wqkv_b, out_w, b, g, cosT, sinT, xT_bf[b]))

    nc = bass.Bass()
    _build_mha(nc)
    _split_multi_waits(nc)

    trace = bool(os.environ.get("MHA_TRACE"))
    if trace:
        # dev-only profiling path; skip the S3 artifact upload
        import concourse.bass_utils as _bu
        _bu.upload_artifacts = lambda tmpdir: tmpdir
    res = run_bass_kernel_spmd(
        nc, in_maps, core_ids=list(range(8)), trace=trace)
    if trace:
        LAST_EXEC_NS = res.exec_time_ns

    out = np.empty((2, S, E), dtype=np.float32)
    for b in range(2):
        acc = res.results[b * 4 + 0]["out"].astype(np.float32)
        for g in range(1, 4):
            acc += res.results[b * 4 + g]["out"].astype(np.float32)
        out[b] = acc + out_b[None, :]
    return out


# revision 52
# speedup vs baseline: 1.0170x; 1.0097x over previous
"""Sharded MHA-with-RoPE Trainium2 kernel (nn_CustomTorchMHASelf).

Contract: kernel(**inputs) takes the FULL unsharded inputs of the
reference (x [2,2048,2048], Wqkv_w [6144,2048], Wqkv_b [6144],
out_w [2048,2048], out_b [2048]) and returns the full [2,2048,2048]
fp32 output, running the compute on 8 NeuronCores.

Sharding: core = b*4 + g handles batch b and head-group g (4 of the 16
heads). Each core computes q/k/v projections for its heads, RoPE,
softmax attention, and its slice of the out-projection; the host sums
the 4 partial outputs per batch and adds out_b.

Device data plane is bf16 (fp32 PSUM accumulation); the host
pre-transposes x and the weight slices into the layouts the TensorE
wants (contraction dim on partitions everywhere).

Schedule: pass 1 computes K+RoPE and V for all tokens (the last block
also hides the attention prologue under its V-projection); pass 2 is a
flat software pipeline over (block, head) steps -- at step k the PE
stream interleaves att@V(k), scores(k+1), q-projection(k+2) and a
quarter of the previous block's out-projection (65 matmuls/step), so
the ScalarE exp stream (16/step) is never on the critical path.
Key device tricks:
  - rotate-half for RoPE is a PE matmul with a signed permutation
    matrix (SBUF-SBUF DMA swaps are slow and their DIRECT2D triggers
    serialize on the sync sequencer);
  - the softmax denominator is a bf16 tree-add into the attB tile on
    VectorE plus ONE ones-matmul per (head, block) instead of 16 full
    PE ones-matmuls; att is split into two tiles (attA/attB) so the
    tree's writes never alias tiles the PE still reads (the dep
    tracker is coarse); the ones-matmul+normalize are deferred one
    step so the PE never waits on the tree;
  - 1/denominator = Exp(-Ln(d)) on ScalarE (ln and exp share an
    activation table, so no table reloads) because DVE reciprocal is
    slow and custom-DVE ops don't compile on this toolchain;
  - ~40 warm-up matmuls on the ones tile ramp the PE out of its
    1.2GHz cold p-state while the first weight/x DMAs land;
  - output tiles are written bf16, with drain-phase DMA triggers
    alternating between the scalar and sync queues.
"""

import math
import os
import sys
import types

import numpy as np
import ml_dtypes

import concourse.bass as bass
import concourse.mybir as mybir
import concourse.tile as tile
from concourse.bass import ds

F32 = mybir.dt.float32
BF16 = mybir.dt.bfloat16
Alu = mybir.AluOpType
Act = mybir.ActivationFunctionType
BF = ml_dtypes.bfloat16

S, E, HTOT, HL, D, P = 2048, 2048, 16, 4, 128, 128

# Filled with the profile exec time (ns) when MHA_TRACE=1; read by test.py.
LAST_EXEC_NS = None


def _install_axon_ntff_shim():
    """Provide antenv.axon_hooks so trace=True can reach the axon NTFF hook."""
    if "antenv.axon_hooks" in sys.modules:
        return
    mod = types.ModuleType("antenv.axon_hooks")
    holder = [None]
    mod.set_axon_ntff_profile_hook = lambda h: holder.__setitem__(0, h)
    mod.get_axon_ntff_profile_hook = lambda: holder[0]
    sys.modules["antenv.axon_hooks"] = mod
    try:
        import antenv
        antenv.axon_hooks = mod
    except ImportError:
        pass
    # boot() ran at interpreter start (sitecustomize), before this module
    # existed, so its NTFF-hook registration was silently skipped. Redo it.
    try:
        from trn_agent_boot.trn_boot import _ntff_profile_via_ctypes
        hook = _ntff_profile_via_ctypes("/opt/axon/libaxon_pjrt.so")
        if hook is not None:
            mod.set_axon_ntff_profile_hook(hook)
    except Exception:
        pass


def _split_multi_waits(nc):
    """Hoist extra sem-waits onto standalone NoOps (one wait per inst).

    This walrus build rejects any instruction carrying more than one
    sync-wait ("Too many sync wait commands"); Tile attaches one wait per
    outstanding semaphore to the consuming instruction. Splitting them
    across same-engine NoOps placed immediately before is equivalent:
    the engine executes serially, so all waits still precede the inst.
    """
    ctr = 0
    for fn in nc.m.functions:
        for blk in fn.blocks:
            out = []
            for inst in blk.instructions:
                si = getattr(inst, "sync_info", None)
                if si is not None and si.on_wait is not None \
                        and len(si.on_wait) > 1:
                    waits = list(si.on_wait)
                    si.on_wait = [waits[-1]]
                    for w in waits[:-1]:
                        ctr += 1
                        nop = mybir.InstNoOp(
                            name=f"I-wsplit-{ctr}", ins=[], outs=[])
                        nop.engine = inst.engine
                        nop.sync_info = mybir.SyncInfo(
                            on_wait=[w], on_update=[])
                        out.append(nop)
                out.append(inst)
            blk.instructions[:] = out


def _build_mha(nc: bass.Bass):
    """Emit the per-core MHA program (one shard) into `nc`."""
    EO = E // P            # contraction subtiles for the projections
    ST = 512               # free-dim tile (one PSUM bank of fp32)
    NS = S // ST
    SB = S // P
    JT = S // P            # key blocks per head
    ET = E // ST
    H = D // 2

    # packed layouts: [.., P, EO, ST] so DMA descriptors are 2KB
    # per-partition runs (1KB rows are descriptor-bound at ~half the
    # per-queue DMA bandwidth)
    xP = nc.dram_tensor("xP", [NS, P, EO, ST], BF16, kind="ExternalInput")
    wkP = nc.dram_tensor("wkP", [P, EO, HL * D], BF16, kind="ExternalInput")
    wqP = nc.dram_tensor("wqP", [P, EO, HL * D], BF16, kind="ExternalInput")
    wvP = nc.dram_tensor("wvP", [P, EO, HL * D], BF16, kind="ExternalInput")
    qkb = nc.dram_tensor("qkb", [2 * HL, D], F32, kind="ExternalInput")
    vb = nc.dram_tensor("vb", [HL * D], F32, kind="ExternalInput")
    cosT = nc.dram_tensor("cosT", [D, S], BF16, kind="ExternalInput")
    sinT = nc.dram_tensor("sinT", [D, S], BF16, kind="ExternalInput")
    owT = nc.dram_tensor("owT", [HL * D, E], BF16, kind="ExternalInput")
    ones = nc.dram_tensor("ones", [P, P], BF16, kind="ExternalInput")
    perm = nc.dram_tensor("perm", [P, P], BF16, kind="ExternalInput")
    out = nc.dram_tensor("out", [S, E], BF16, kind="ExternalOutput")

    isc = 1.0 / math.sqrt(D)

    from contextlib import ExitStack

    with tile.TileContext(nc) as tc, ExitStack() as stk:
        persist = stk.enter_context(tc.tile_pool(name="persist", bufs=1))
        kT_sb = persist.tile([P, HL, S], BF16)      # k post-RoPE [d, h, s]
        v_sb = persist.tile([P, SB, HL * D], BF16)  # v natural [s%128, s//128, hd]
        ctxT_sb = persist.tile([P, HL, S], BF16)    # [d, h, i]
        ones_sb = persist.tile([P, P], BF16)
        perm_sb = persist.tile([P, P], BF16)
        cos_sb = persist.tile([P, S], BF16)
        sin_sb = persist.tile([P, S], BF16)
        qkb_sb = persist.tile([P, 2 * HL], F32)
        vb_sb = persist.tile([P, HL * D], F32)
        ow_sb = persist.tile([P, HL, E], BF16)
        nc.sync.dma_start(ones_sb[:], ones[:])
        nc.sync.dma_start(qkb_sb[:], qkb[:].rearrange("c d -> d c"))
        nc.sync.dma_start(perm_sb[:], perm[:])

        # x stream shared by both passes; rope temps likewise.  qb/rot are
        # still being read (by the swap DMAs / mults) when the next rope
        # starts, so they get 2 bufs; t1/t2 are consumed immediately by the
        # in-order VectorE queue, so 1 buf suffices.
        xs = stk.enter_context(tc.tile_pool(name="xstream", bufs=2))
        rta = stk.enter_context(tc.tile_pool(name="ropetmpa", bufs=4))
        rtb = stk.enter_context(tc.tile_pool(name="ropetmpb", bufs=1))
        wqp = stk.enter_context(tc.tile_pool(name="wqpool", bufs=1))
        wq_sb = wqp.tile([P, EO, HL * D], BF16)

        psA = stk.enter_context(tc.tile_pool(name="psA", bufs=4, space="PSUM"))
        psS = stk.enter_context(tc.tile_pool(name="psS", bufs=2, space="PSUM"))
        psC = stk.enter_context(tc.tile_pool(name="psC", bufs=2, space="PSUM"))

        qp = stk.enter_context(tc.tile_pool(name="qpool", bufs=4))
        dp = stk.enter_context(tc.tile_pool(name="denp", bufs=1))
        oc = stk.enter_context(tc.tile_pool(name="ocopy", bufs=6))
        at0 = stk.enter_context(tc.tile_pool(name="att0p", bufs=1))

        # flat (block, head) schedule for the attention pass; blocks in
        # reverse order so the first one reuses pass 1's last x tile
        order = list(range(NS - 1, -1, -1))
        seq = [(i, h) for i in order for h in range(HL)]
        NK = len(seq)

        def blk(k):
            return seq[k][0]

        def sl_of(k):
            return ds(blk(k) * ST, ST)

        def dma_packed(dst, srcap):
            # dst [P, EO, ST] SBUF tile, srcap [P, EO, ST] DRAM view with
            # per-partition-contiguous rows: 16 transfers of 64x2KB descs
            for j in range(EO // 2):
                for ph in range(2):
                    nc.sync.dma_start(
                        dst[ds(ph * 64, 64), ds(2 * j, 2), :],
                        srcap[ds(ph * 64, 64), ds(2 * j, 2), :])

        def rope_begin(ps, bias_ap):
            # qb = q + bias (bf16 so the rotate-half matmul runs full rate)
            qb = rta.tile([P, ST], BF16, tag="qb")
            nc.vector.tensor_scalar_add(qb[:], ps[:], bias_ap)
            return qb

        def rope_finish(qb, sl, out_ap):
            # rotate-half as a PE matmul with a signed permutation matrix
            # (cross-partition moves otherwise need a slow SBUF-SBUF DMA
            # whose trigger also serializes on the sync sequencer);
            # out = qb*cos + (perm.T @ qb)*sin.
            rps = psS.tile([P, ST], F32, tag="sc")
            nc.tensor.matmul(rps[:], perm_sb[:], qb[:], start=True, stop=True)
            t1 = rtb.tile([P, ST], BF16, tag="t1")
            t2 = rtb.tile([P, ST], BF16, tag="t2")
            nc.vector.tensor_tensor(t1[:], qb[:], cos_sb[:, sl], Alu.mult)
            nc.vector.tensor_tensor(t2[:], rps[:], sin_sb[:, sl], Alu.mult)
            nc.vector.tensor_tensor(out_ap, t1[:], t2[:], Alu.add)

        def qproj_mm(k, psq, xt, eo):
            h = seq[k][1]
            nc.tensor.matmul(
                psq[:], wq_sb[:, eo, ds(h * D, D)], xt[:, eo, :],
                start=(eo == 0), stop=(eo == EO - 1))

        def qproj_fin(k, psq):
            qt = qp.tile([P, ST], BF16, tag="qt")
            qb = rope_begin(psq, qkb_sb[:, HL + seq[k][1], None])
            return qt, (qb, sl_of(k), qt[:])

        def scores_mm(k, attab, qt, jb):
            h = seq[k][1]
            ps = psS.tile([P, ST], F32, tag="sc")
            nc.tensor.matmul(
                ps[:], kT_sb[:, h, ds(jb * P, P)], qt[:],
                start=True, stop=True)
            dst = attab[jb // 8][:, jb % 8, :]
            nc.scalar.activation(dst, ps[:], Act.Exp, scale=isc)

        # PE warm-up: matmuls on the just-landed ones tile so the PE ramps
        # to full clock while the big weight/x DMAs stream in (cold matmuls
        # otherwise run at the 1.2GHz mid p-state for ~10us)
        wsink = persist.tile([P, 1], F32)
        wps = psS.tile([P, ST], F32, tag="sc", name="warmps")
        NWARM = 68
        for w in range(NWARM):
            nc.tensor.matmul(
                wps[:, :P], ones_sb[:], ones_sb[:],
                start=(w == 0), stop=(w == NWARM - 1))
        nc.vector.tensor_copy(wsink[:], wps[:, :1])

        # ---- pass 1: K projection + RoPE, V projection ----
        # The last block additionally hides the attention pass's prologue
        # (q heads 0/1 + scores/exp for head 0) under its V-projection.
        xt_last = None
        att0 = None
        qtiles = []
        with tc.tile_pool(name="p1w", bufs=1) as p1:
            wk_sb = p1.tile([P, EO, HL * D], BF16)
            wv_sb = p1.tile([P, EO, HL * D], BF16)
            # DMA priority order: the first K matmul group needs all of
            # xt0 + wk; cos/sin (bf16, 1MB) unblock the first RoPEs; then
            # x block 1, wv (needed ~25us in), and the small tables.
            xt0 = xs.tile([P, EO, ST], BF16, tag="xt", name="xt0")
            for j in range(EO // 2):
                for ph in range(2):
                    nc.sync.dma_start(
                        wk_sb[ds(ph * 64, 64), ds(2 * j, 2), :],
                        wkP[ds(ph * 64, 64), ds(2 * j, 2), :])
                    nc.sync.dma_start(
                        xt0[ds(ph * 64, 64), ds(2 * j, 2), :],
                        xP[0][ds(ph * 64, 64), ds(2 * j, 2), :])
            dma_packed(wv_sb, wvP[:])
            xt1 = xs.tile([P, EO, ST], BF16, tag="xt", name="xt1")
            dma_packed(xt1, xP[1])
            # cos/sin feed only the VectorE side of RoPE, which can lag;
            # nothing PE-side waits on them
            nc.sync.dma_start(cos_sb[:], cosT[:])
            nc.sync.dma_start(sin_sb[:], sinT[:])
            nc.sync.dma_start(vb_sb[:], vb[None, :].to_broadcast((P, HL * D)))

            for i in range(NS):
                if i == 0:
                    xt = xt0
                elif i == 1:
                    xt = xt1
                else:
                    xt = xs.tile([P, EO, ST], BF16, tag="xt")
                    dma_packed(xt, xP[i])
                    if i == NS - 1:
                        # prefetch pass-2 weights behind this block's x:
                        # wq feeds the q-projections later in this block,
                        # ow the out-projection a block later
                        dma_packed(wq_sb, wqP[:])
                        for ho in range(HL):
                            nc.sync.dma_start(
                                ow_sb[:, ho, :], owT[ds(ho * P, P), :])
                sl = ds(i * ST, ST)
                kropes = []
                for jb in range(HL):       # k head jb
                    ps = psA.tile([P, ST], F32, tag="acc")
                    for eo in range(EO):
                        nc.tensor.matmul(
                            ps[:], wk_sb[:, eo, ds(jb * D, D)], xt[:, eo, :],
                            start=(eo == 0), stop=(eo == EO - 1))
                    qb = rope_begin(ps, qkb_sb[:, jb, None])
                    kropes.append((qb, sl, kT_sb[:, jb, sl]))
                if i < NS - 1:
                    for sbl in range(ST // P):
                        sb = i * (ST // P) + sbl
                        ps = psS.tile([P, ST], F32, tag="sc")
                        for eo in range(EO):
                            nc.tensor.matmul(
                                ps[:, : HL * D], xt[:, eo, ds(sbl * P, P)],
                                wv_sb[:, eo, :],
                                start=(eo == 0), stop=(eo == EO - 1))
                        nc.vector.tensor_tensor(
                            v_sb[:, sb, :], ps[:, : HL * D], vb_sb[:], Alu.add)
                    for kr in kropes:
                        rope_finish(*kr)
                else:
                    for kr in kropes:
                        rope_finish(*kr)
                    # q-projections for the first two attention heads, then
                    # V-projection interleaved with scores/exp for head 0
                    psq0 = psA.tile([P, ST], F32, tag="acc")
                    for eo in range(EO):
                        qproj_mm(0, psq0, xt, eo)
                    qt0, rf0 = qproj_fin(0, psq0)
                    psq1 = psA.tile([P, ST], F32, tag="acc")
                    for eo in range(EO):
                        qproj_mm(1, psq1, xt, eo)
                    qt1, rf1 = qproj_fin(1, psq1)
                    qtiles.extend([qt0, qt1])
                    rope_finish(*rf0)
                    att0 = (at0.tile([P, 8, ST], BF16, name="att0A"),
                            at0.tile([P, 8, ST], BF16, name="att0B"))
                    sc_jb = 0
                    vps = None
                    for vi in range(4 * EO):
                        sbl, eo = vi // EO, vi % EO
                        if eo == 0:
                            vps = psA.tile([P, ST], F32, tag="acc")
                        nc.tensor.matmul(
                            vps[:, : HL * D], xt[:, eo, ds(sbl * P, P)],
                            wv_sb[:, eo, :],
                            start=(eo == 0), stop=(eo == EO - 1))
                        if eo == EO - 1:
                            nc.vector.tensor_tensor(
                                v_sb[:, i * (ST // P) + sbl, :],
                                vps[:, : HL * D], vb_sb[:], Alu.add)
                        if vi >= EO and (vi - EO) % 3 == 0 and sc_jb < JT:
                            scores_mm(0, att0, qt0, sc_jb)
                            sc_jb += 1
                        if vi == 40:
                            rope_finish(*rf1)
                if i == NS - 1:
                    xt_last = xt

        # ---- pass 2: flat software pipeline over (block, head) steps ----
        # Blocks run in reverse so the first one reuses pass 1's last x
        # tile.  At step k: att@V + denominator tree for head k, scores+exp
        # for head k+1, Q-projection for head k+2, one quarter of the
        # PREVIOUS block's out-projection, and the deferred denominator
        # ones-matmul + normalize for head k-1 -- all interleaved so the PE
        # stream (65 matmuls/step) hides the exp stream (16/step).
        with tc.tile_pool(name="attpa", bufs=3) as abA, \
             tc.tile_pool(name="attpb", bufs=3) as abB:

            xts = {order[0]: xt_last}

            def cblock_mm(ci, jb, pst, drain=False):
                # one of the 16x4 out-projection matmuls for token block ci;
                # jb runs 0..63 across the block's four steps
                tile_i, ho = jb // HL, jb % HL
                sb = ci * (ST // P) + tile_i // ET
                et = tile_i % ET
                if ho == 0:
                    pst[0] = psC.tile([P, ST], F32, tag="ct", name="ct")
                nc.tensor.matmul(
                    pst[0][:], ctxT_sb[:, ho, ds(sb * P, P)],
                    ow_sb[:, ho, ds(et * ST, ST)],
                    start=(ho == 0), stop=(ho == HL - 1))
                if ho == HL - 1:
                    ot = oc.tile([P, ST], BF16, tag="ot")
                    nc.vector.tensor_copy(ot[:], pst[0][:])
                    # during the drain the ScalarE queue is idle (no exp
                    # stream): alternate triggers between it and the sync
                    # queue so neither serializes the tail; split the last
                    # tiles' transfers so the final DMA is short
                    if not drain:
                        nc.sync.dma_start(
                            out[ds(sb * P, P), ds(et * ST, ST)], ot[:])
                    else:
                        eng = nc.scalar if (tile_i % 2) else nc.sync
                        if tile_i < 12:
                            eng.dma_start(
                                out[ds(sb * P, P), ds(et * ST, ST)], ot[:])
                        else:
                            hst = ST // 2
                            eng.dma_start(
                                out[ds(sb * P, P), ds(et * ST, hst)],
                                ot[:, :hst])
                            eng2 = nc.sync if (tile_i % 2) else nc.scalar
                            eng2.dma_start(
                                out[ds(sb * P, P), ds(et * ST + hst, hst)],
                                ot[:, hst:])

            def finish(k, attab, psc):
                # denominator ones-matmul + normalize for head k; deferred
                # one step so the PE reaches the ones-matmul well after the
                # VectorE tree produced attB[:, 0, :].  1/d = Exp(-Ln(d)) on
                # ScalarE (ln/exp share an activation table: no reloads).
                psd = psC.tile([P, ST], F32, tag="ct")
                nc.tensor.matmul(
                    psd[:], ones_sb[:], attab[1][:, 0, :],
                    start=True, stop=True)
                lnd = dp.tile([P, ST], F32, tag="lnd")
                nc.scalar.activation(lnd[:], psd[:], Act.Ln)
                rec = dp.tile([P, ST], F32, tag="rec")
                nc.scalar.activation(rec[:], lnd[:], Act.Exp, scale=-1.0)
                nc.vector.tensor_tensor(
                    ctxT_sb[:, seq[k][1], sl_of(k)], psc[:], rec[:], Alu.mult)

            atts = [att0]

            cpst = [None]
            pending = None
            for k in range(NK):
                i, h = seq[k]
                if h == 0 and k + 4 < NK:
                    # prefetch the x tile for the NEXT block now; the DMA
                    # has a whole block (~55us) to land
                    nxt = blk(k + 4)
                    xtn = xs.tile([P, EO, ST], BF16, tag="xt")
                    dma_packed(xtn, xP[nxt])
                    xts[nxt] = xtn
                att = atts[k]
                attA, attB = att
                ci_prev = blk(k - 4) if k >= 4 else None
                if k + 1 < NK:
                    attn = (abA.tile([P, 8, ST], BF16, tag="attA", name="attA"),
                            abB.tile([P, 8, ST], BF16, tag="attB", name="attB"))
                    atts.append(attn)
                else:
                    attn = None
                if k + 2 < NK:
                    psq = psA.tile([P, ST], F32, tag="acc")
                else:
                    psq = None
                psc = psA.tile([P, ST], F32, tag="acc")
                for idx in range(JT):
                    if attn is not None:
                        scores_mm(k + 1, attn, qtiles[k + 1], idx)
                    jb = (idx + 8) % JT     # att@V: B half first
                    avs = (attA, attB)[jb // 8][:, jb % 8, :]
                    nc.tensor.matmul(
                        psc[:], v_sb[:, jb, ds(h * D, D)], avs,
                        start=(idx == 0), stop=(idx == JT - 1))
                    if psq is not None:
                        qproj_mm(k + 2, psq, xts[blk(k + 2)], idx)
                    if ci_prev is not None:
                        if k % 4 != 0:
                            cblock_mm(ci_prev, (k % 4) * JT + idx, cpst)
                        elif idx >= 8:
                            # block-boundary step: the previous block's ctx
                            # normalize lands ~1us in, so weave its out-
                            # projection into the back half, two per slot
                            cblock_mm(ci_prev, (idx - 8) * 2, cpst)
                            cblock_mm(ci_prev, (idx - 8) * 2 + 1, cpst)
                    # denominator tree levels woven into the matmul
                    # stream; they only ever write attB, whose att@V reads
                    # all finished at idx 7
                    if idx == 7:
                        nc.vector.tensor_tensor(
                            attB[:], attB[:], attA[:], Alu.add)
                    elif idx == 11:
                        nc.vector.tensor_tensor(
                            attB[:, 0:4, :], attB[:, 0:4, :],
                            attB[:, 4:8, :], Alu.add)
                    elif idx == 13:
                        nc.vector.tensor_tensor(
                            attB[:, 0:2, :], attB[:, 0:2, :],
                            attB[:, 2:4, :], Alu.add)
                    elif idx == 15:
                        nc.vector.tensor_tensor(
                            attB[:, 0, :], attB[:, 0, :], attB[:, 1, :],
                            Alu.add)
                if psq is not None:
                    qt, rf = qproj_fin(k + 2, psq)
                    qtiles.append(qt)
                else:
                    rf = None
                if pending is not None:
                    finish(*pending)
                    pending = None
                if h == HL - 1:
                    # last head of the block: finish NOW so the next
                    # block's interleaved out-projection reads final ctx
                    finish(k, att, psc)
                else:
                    pending = (k, att, psc)
                if rf is not None:
                    rope_finish(*rf)

            # the last block's out-projection has no next block to hide in
            cpst = [None]
            for jb in range(4 * JT):
                cblock_mm(blk(NK - 1), jb, cpst, drain=True)

    return nc


def _rope_tables():
    inv_freq = 1.0 / (10000.0 ** (np.arange(0, D, 2, dtype=np.float32) / D))
    t = np.arange(S, dtype=np.float32)
    freqs = np.einsum("s,f->sf", t, inv_freq)
    emb = np.concatenate([freqs, freqs], axis=-1)
    cosT = np.cos(emb).astype(np.float32).T.copy()
    sinT = np.sin(emb).astype(np.float32).T.copy()
    # rotate-half sign lives in the on-device permutation matrix
    return cosT.astype(BF), np.ascontiguousarray(sinT).astype(BF)


def _core_inputs(x, Wqkv_w, Wqkv_b, out_w, b, g, cosT, sinT, xT_bf):
    # k-head columns first, then q-head columns (matches kernel layout)
    k_cols, q_cols, kb_rows, qb_rows = [], [], [], []
    for hl in range(HL):
        h = g * HL + hl
        q_cols.append(Wqkv_w[h * D:(h + 1) * D, :].T)
        k_cols.append(Wqkv_w[E + h * D:E + (h + 1) * D, :].T)
        qb_rows.append(Wqkv_b[h * D:(h + 1) * D])
        kb_rows.append(Wqkv_b[E + h * D:E + (h + 1) * D])
    def pack(wT):
        # [E, HL*D] -> [P, EO, HL*D]: per-partition contiguous rows so
        # the on-device DMA uses 2KB descriptors
        return np.ascontiguousarray(
            wT.reshape(E // P, P, HL * D).transpose(1, 0, 2)).astype(BF)

    wkP = pack(np.concatenate(k_cols, axis=1))
    wqP = pack(np.concatenate(q_cols, axis=1))
    qkb = np.stack(kb_rows + qb_rows).astype(np.float32)
    v0 = 2 * E + g * HL * D
    wvP = pack(Wqkv_w[v0:v0 + HL * D, :].T)
    vb = Wqkv_b[v0:v0 + HL * D].astype(np.float32)
    owT = np.ascontiguousarray(
        out_w[:, g * HL * D:(g + 1) * HL * D].T).astype(BF)
    # rotate-half permutation: out[d] = -q[d+64] (d<64), +q[d-64] (d>=64)
    perm = np.zeros((P, P), dtype=np.float32)
    for d in range(D // 2):
        perm[d + D // 2, d] = -1.0
        perm[d, d + D // 2] = 1.0
    return {"xP": xT_bf, "wkP": wkP, "wqP": wqP, "wvP": wvP, "qkb": qkb,
            "vb": vb, "cosT": cosT, "sinT": sinT, "owT": owT,
            "ones": np.ones((P, P), BF), "perm": perm.astype(BF)}


def kernel(x, Wqkv_w, Wqkv_b, out_w, out_b):
    global LAST_EXEC_NS
    _install_axon_ntff_shim()
    from concourse.bass_utils import run_bass_kernel_spmd

    x = np.asarray(x, dtype=np.float32)
    Wqkv_w = np.asarray(Wqkv_w, dtype=np.float32)
    Wqkv_b = np.asarray(Wqkv_b, dtype=np.float32)
    out_w = np.asarray(out_w, dtype=np.float32)
    out_b = np.asarray(out_b, dtype=np.float32)

    cosT, sinT = _rope_tables()
    # x packed as [NS, P, EO, ST]: xP[i, p, eo, s] = x[b, i*ST+s, eo*P+p]
    NS, EO, ST = S // 512, E // P, 512
    xT_bf = [np.ascontiguousarray(
        x[b].reshape(NS, ST, EO, P).transpose(0, 3, 2, 1)).astype(BF)
        for b in range(2)]
    in_maps = []
    for core in range(8):
        b, g = core // 4, core % 4
        in_maps.append(
            _core_inputs(x, Wqkv_w, Wqkv_b, out_w, b, g, cosT, sinT, xT_bf[b]))

    nc = bass.Bass()
    _build_mha(nc)
    _split_multi_waits(nc)

    trace = bool(os.environ.get("MHA_TRACE"))
    if trace:
        # dev-only profiling path; skip the S3 artifact upload
        import concourse.bass_utils as _bu
        _bu.upload_artifacts = lambda tmpdir: tmpdir
    res = run_bass_kernel_spmd(
        nc, in_maps, core_ids=list(range(8)), trace=trace)
    if trace:
        LAST_EXEC_NS = res.exec_time_ns

    out = np.empty((2, S, E), dtype=np.float32)
    for b in range(2):
        acc = res.results[b * 4 + 0]["out"].astype(np.float32)
        for g in range(1, 4):
            acc += res.results[b * 4 + g]["out"].astype(np.float32)
        out[b] = acc + out_b[None, :]
    return out
